# revision 1
# baseline (speedup 1.0000x reference)
"""Trainium2 Bass kernel for nn_CustomClassificationLoss_48765058678812.

Loss (see reference): per sample b with target t, each class c at circular
distance d(c,t) = min((c-t)%360, (t-c)%360) contributes |0.98**d - x[b,c]|
(d=0 gives 1-x, valid since x in [0,1)), except d == 180 contributes 0.
loss = sum over all (b, c) / B.

Pure data parallel over 8 cores (8192 samples each). Per 128-sample group,
with per-partition scalar t (one sample per partition):
  delta = iota - t                          (DVE tensor_scalar, fp16)
  a = |delta|   via int16 AND 0x7fff        (fp16 sign clear)
  b = |a - 180|                             (= 180 - d; two more TS ops)
  w = Exp(-ln(.98)*b + 180*ln(.98)) on ACT  (= 0.98^d, the profile row)
  m = min(b, 1)                             (0 iff d == 180 -> the mask)
  acc[p] += sum_c |m * (w - x)|             (TT sub, TT mul, ACT Abs+accum)
Host sums the per-core [128, 64] partials and divides by B.

Two workarounds for the pinned neuronxcc build (allows at most ONE sem-wait
per instruction and rejects the EVENT_SEMAPHORE_RANGE_CLEAR ISA blob):
  - clear_and_free_semaphores: skip the gpsimd dma_reset/sem_clear tail ops
    (bookkeeping kept; repeat executions validated on hardware)
  - _split_multi_waits: post-pass hoisting extra sem-waits onto injected NoOps
"""

import math
import numpy as np
from contextlib import ExitStack

import concourse.bass as bass
import concourse.tile as tile
from concourse import mybir
from concourse.bass_utils import run_bass_kernel_spmd

NUM_CLASSES = 360
DECAY = 0.98
LN98 = math.log(DECAY)
N_CORES = 8
B_TOTAL = 65536
B_SHARD = B_TOTAL // N_CORES        # 8192
GROUPS = B_SHARD // 128             # 64 groups of 128 samples
CHUNK_GROUPS = 8                    # groups per X-load chunk

_CACHE: dict = {}


def _patched_clear_and_free_semaphores(self, sems):
    # The pinned walrus rejects the EVENT_SEMAPHORE_RANGE_CLEAR InstISA the
    # stock implementation emits; keep only the allocator bookkeeping.
    if not sems:
        return
    sem_nums = [s.num if hasattr(s, "num") else s for s in sems]
    self._state.prepend_free_semaphores(sem_nums)
    for poison_set in self._tile_sem_poison_stack:
        poison_set.update(sem_nums)


def _split_multi_waits(nc):
    # The pinned walrus accepts at most one sem-wait per instruction; hoist
    # extras onto same-engine NoOps placed immediately before.
    for f in nc.m.functions:
        for b in f.blocks:
            out = []
            changed = False
            for ins in b.instructions:
                si = ins.sync_info
                waits = list(si.on_wait) if (si and si.on_wait) else []
                if len(waits) > 1 and ins.engine is not None:
                    for j, w in enumerate(waits[:-1]):
                        nop = mybir.InstNoOp(
                            name=f"{ins.name}_hw{j}", engine=ins.engine,
                            ins=[], outs=[],
                        )
                        nop.sync_info = mybir.SyncInfo(on_wait=[w], on_update=[])
                        nc.register_instruction(nop)
                        out.append(nop)
                    si.on_wait = [waits[-1]]
                    changed = True
                out.append(ins)
            if changed:
                b.instructions = out


def _build_nc() -> bass.Bass:
    bass.Bass.clear_and_free_semaphores = _patched_clear_and_free_semaphores
    nc = bass.Bass()
    f16 = mybir.dt.float16
    f32 = mybir.dt.float32
    i16 = mybir.dt.int16
    logits = nc.dram_tensor(
        "logits", [B_SHARD, NUM_CLASSES], f32, kind="ExternalInput"
    )
    tidx = nc.dram_tensor("tidx", [128, GROUPS], mybir.dt.int32, kind="ExternalInput")
    partial = nc.dram_tensor("partial", [128, GROUPS], f32, kind="ExternalOutput")

    # sample b = n*128 + p -> partition p, group n
    logits_r = logits.rearrange("(n p) c -> p n c", p=128)

    with tile.TileContext(nc) as tc, ExitStack() as ctx:
        singles = ctx.enter_context(tc.tile_pool(name="singles", bufs=1))
        xpool = ctx.enter_context(tc.tile_pool(name="xpool", bufs=3))
        wpool = ctx.enter_context(tc.tile_pool(name="wpool", bufs=3))
        tpool = ctx.enter_context(tc.tile_pool(name="tpool", bufs=3))

        idx_sb = singles.tile([128, GROUPS], mybir.dt.int32)
        nc.sync.dma_start(out=idx_sb, in_=tidx[:, :])
        tf32 = singles.tile([128, GROUPS], f32)
        nc.vector.tensor_copy(out=tf32, in_=idx_sb)
        iota_i = singles.tile([128, NUM_CLASSES], i16)
        nc.gpsimd.iota(iota_i, pattern=[[1, NUM_CLASSES]], base=0, channel_multiplier=0)
        iota_f = singles.tile([128, NUM_CLASSES], f16)
        nc.vector.tensor_copy(out=iota_f, in_=iota_i)
        accbuf = singles.tile([128, GROUPS], f32)
        exp_bias = singles.tile([128, 1], f32)
        nc.vector.memset(exp_bias, 180.0 * LN98)

        for i in range(GROUPS // CHUNK_GROUPS):
            g0 = i * CHUNK_GROUPS
            xt = xpool.tile([128, CHUNK_GROUPS, NUM_CLASSES], f16, tag="xt")
            nc.gpsimd.dma_start(
                out=xt, in_=logits_r[:, g0 : g0 + CHUNK_GROUPS, :]
            )
            for g in range(CHUNK_GROUPS):
                gg = g0 + g
                bt = wpool.tile([128, NUM_CLASSES], f16, tag="bt")
                nc.vector.tensor_scalar(
                    out=bt, in0=iota_f, scalar1=tf32[:, gg : gg + 1], scalar2=None,
                    op0=mybir.AluOpType.subtract)
                nc.vector.tensor_scalar(
                    out=bt.bitcast(i16), in0=bt.bitcast(i16), scalar1=0x7FFF,
                    scalar2=None, op0=mybir.AluOpType.bitwise_and)
                nc.vector.tensor_scalar(
                    out=bt, in0=bt, scalar1=180.0, scalar2=None,
                    op0=mybir.AluOpType.subtract)
                nc.vector.tensor_scalar(
                    out=bt.bitcast(i16), in0=bt.bitcast(i16), scalar1=0x7FFF,
                    scalar2=None, op0=mybir.AluOpType.bitwise_and)
                wt = tpool.tile([128, NUM_CLASSES], f16, tag="wt")
                nc.scalar.activation(
                    out=wt, in_=bt, func=mybir.ActivationFunctionType.Exp,
                    scale=-LN98, bias=exp_bias)
                # m = min(b, 1) in place on bt
                nc.vector.tensor_scalar(
                    out=bt, in0=bt, scalar1=1.0, scalar2=None,
                    op0=mybir.AluOpType.min)
                nc.vector.tensor_sub(out=wt, in0=wt, in1=xt[:, g, :])
                nc.vector.tensor_mul(out=wt, in0=wt, in1=bt)
                at = tpool.tile([128, NUM_CLASSES], f16, tag="at")
                nc.scalar.activation(
                    out=at, in_=wt, func=mybir.ActivationFunctionType.Abs,
                    accum_out=accbuf[:, gg : gg + 1])

        nc.sync.dma_start(out=partial[:, :], in_=accbuf)

    _split_multi_waits(nc)
    nc.finalize()
    return nc


def _get_nc() -> bass.Bass:
    if "nc" not in _CACHE:
        _CACHE["nc"] = _build_nc()
    return _CACHE["nc"]


def _prep_in_maps(logits: np.ndarray, targets: np.ndarray) -> list[dict]:
    in_maps = []
    for core in range(N_CORES):
        sl = slice(core * B_SHARD, (core + 1) * B_SHARD)
        t = np.ascontiguousarray(targets[sl]).astype(np.int32)
        # sample b = g*128 + p -> idx[p, g]
        idx = np.ascontiguousarray(t.reshape(GROUPS, 128).T)
        in_maps.append({"logits": np.ascontiguousarray(logits[sl]), "tidx": idx})
    return in_maps


def kernel(logits, targets):
    logits = np.asarray(logits, dtype=np.float32)
    targets_np = np.asarray(targets).astype(np.int64)
    assert logits.shape == (B_TOTAL, NUM_CLASSES), logits.shape
    assert targets_np.shape == (B_TOTAL,), targets_np.shape

    nc = _get_nc()
    in_maps = _prep_in_maps(logits, targets_np)
    res = run_bass_kernel_spmd(nc, in_maps, core_ids=list(range(N_CORES)))
    total = np.float64(0.0)
    for out_map in res.results:
        total += np.asarray(out_map["partial"], np.float64).sum()
    loss = np.float32(total / B_TOTAL)
    return (loss, 0.0, loss)

